# revision 55
# baseline (speedup 1.0000x reference)
"""Trainium2 Bass kernel for nn_AlignModule (QAConv correlation + PAM).

Reference computation (B=32, C=512, H=24, W=8, hw=192, C8=64):
  xf = x.reshape(B, C, hw)
  score[g,p,n,m] = sum_c xf[g,c,m] * xf[p,c,n]          # [B,B,hw,hw]
  kernel_max[g,p,n] = max_m score[g,p,n,m]              # [B,B,hw]
  q = Wq @ xf[b] + bq; k = Wk @ xf[b] + bk              # [B,C8,hw]
  energy[b,m,n] = sum_q q[b,q,m] k[b,q,n]
  pos_max[b,m] = max_n energy[b,m,n]                    # [B,hw]
  out = concat([kernel_max, pos_max[None]], axis=0)     # [B+1,B,hw]

Sharding: data-parallel over g across 8 cores (4 images per core). Each
core gets the full x as [C, B*hw] fp8(e4m3), rolled so its own 4 images
occupy columns [0, 768).

Speed strategy vs the fp32r baseline (84.4us):
 - All QAConv/projection matmuls run in fp8(e4m3) with DoubleRow perf
   mode: one instruction consumes 256 contraction rows at 0.5 cycles
   per moving column -> 4x the fp32r matmul throughput. Tolerance
   allows it: rel err vs output absmax is ~6.8e-3 < 2e-2 (the six
   score tiles holding g==p diagonal blocks accumulate an extra fp8
   hi*lo + lo*hi cross-term to get there).
 - The max-reduction over the score (4.7M fp32 psum elements per core)
   is the real bottleneck; it is spread over DVE and Act (GPSIMD can
   do no elementwise work and nothing may read two PSUM operands):
     D: DVE reduce_max straight from psum (fp32, no perf mode)
     A: Act copy/cast psum -> sbuf fp16, then a DVE tensor_max funnel
        tree (2x perf mode) batched over up to 3 adjacent j-blocks
 - PSUM rotates as 2-bank single-j generations (4 in flight) so banks
   release at fine granularity and the PE rarely stalls; the PAM
   projection / energy generations share the same rotation.
 - A dummy-matmul warmup ramps the PE p-state while the x DMAs fly.
 - No on-chip transposes: outputs are stored reduction-friendly and
   the host de-interleaves.
"""

import numpy as np
import ml_dtypes

import concourse.mybir as mybir
import concourse.tile as tile
from concourse import bacc
from concourse.bass_utils import run_bass_kernel_spmd

B = 32
C = 512
HW = 192
C8 = 64
N_CORES = 8
GPC = B // N_CORES            # images per core (4)
FLAT = B * HW                 # flattened (p, n) axis (6144)
NJ = FLAT // 128              # stationary 128-column blocks (48)
GROLL = GPC * HW              # per-core roll step (768)
NCH = FLAT // GROLL           # x column chunks (8)
JPC = GROLL // 128            # j blocks per column chunk (6)

F32 = mybir.dt.float32
F16 = mybir.dt.float16
F8 = mybir.dt.float8e4
AX_X = mybir.AxisListType.X
DR = mybir.MatmulPerfMode.DoubleRow
IDENT = mybir.ActivationFunctionType.Identity

# D = DVE direct reduce from psum; everything else: Act cast + batched
# DVE tensor_max funnel tree. D j-blocks are spaced so DVE always has
# psum work that does not chain through Act, and so the A-runs between
# them are j-adjacent (batchable).
_D_J = {6, 10, 0, 4, 12, 16, 20, 24, 28, 32, 36, 40, 44}

# tiles (j, gp) that contain g==p diagonal blocks (rolled layout puts
# the core's own images at columns [0, 768) -> j 0..5)
DIAG_TILES = {(0, 0), (1, 0), (2, 0), (3, 1), (4, 1), (5, 1)}

_COMPILED = None


def _build():
    nc = bacc.Bacc("TRN2", target_bir_lowering=False, debug=False)

    xr = nc.dram_tensor("xr", [C, FLAT], F8, kind="ExternalInput").ap()
    xlo = nc.dram_tensor("xlo", [C, GROLL], F8, kind="ExternalInput").ap()
    wq = nc.dram_tensor("wq", [C, C8], F8, kind="ExternalInput").ap()
    wk = nc.dram_tensor("wk", [C, C8], F8, kind="ExternalInput").ap()
    bq = nc.dram_tensor("bq", [C8, 1], F32, kind="ExternalInput").ap()
    bk = nc.dram_tensor("bk", [C8, 1], F32, kind="ExternalInput").ap()
    kres = nc.dram_tensor("kres", [128, NJ, 2, 2], F16, kind="ExternalOutput").ap()
    pam = nc.dram_tensor("pam", [128, 2 * GPC], F16, kind="ExternalOutput").ap()

    xrr = xr.rearrange("(co p) f -> p co f", p=128)
    with tile.TileContext(nc) as tc:
        with (
            tc.tile_pool(name="sb", bufs=1) as sb,
            tc.tile_pool(name="cpool", bufs=8) as cpool,
            tc.tile_pool(name="tpool", bufs=2) as tpool,
            tc.tile_pool(name="psum", bufs=4, space="PSUM") as psum,
        ):
            # ---- input DMAs ----
            xc = [None] * NCH
            xc[0] = sb.tile([128, 4, GROLL], F8, tag="x0", name="x0")
            nc.sync.dma_start(xc[0][:], xrr[:, :, 0:GROLL])
            xc[1] = sb.tile([128, 4, GROLL], F8, tag="x1", name="x1")
            nc.sync.dma_start(xc[1][:], xrr[:, :, GROLL:2 * GROLL])
            wq_sb = sb.tile([128, 4, C8], F8, tag="wq", name="wq_sb")
            nc.sync.dma_start(wq_sb[:], wq.rearrange("(co p) q -> p co q", p=128))
            wk_sb = sb.tile([128, 4, C8], F8, tag="wk", name="wk_sb")
            nc.sync.dma_start(wk_sb[:], wk.rearrange("(co p) q -> p co q", p=128))
            xlo_sb = sb.tile([128, 4, GROLL], F8, tag="xlo", name="xlo_sb")
            nc.sync.dma_start(xlo_sb[:], xlo.rearrange("(co p) f -> p co f", p=128))
            bq_sb = sb.tile([C8, 1], F32, tag="bq", name="bq_sb")
            nc.sync.dma_start(bq_sb[:], bq[:])
            bk_sb = sb.tile([C8, 1], F32, tag="bk", name="bk_sb")
            nc.sync.dma_start(bk_sb[:], bk[:])
            for c in range(2, NCH):
                t = sb.tile([128, 4, GROLL], F8, tag=f"x{c}", name=f"x{c}")
                nc.sync.dma_start(t[:], xrr[:, :, c * GROLL:(c + 1) * GROLL])
                xc[c] = t

            # ---- persistent sbuf ----
            qk_sb = sb.tile([C8, 2, GPC * HW + C8], F16, tag="qk", name="qk_sb")
            res_sb = sb.tile([128, NJ, 2, 2], F16, tag="res", name="res_sb")
            pam_sb = sb.tile([128, 2 * GPC], F16, tag="pam", name="pam_sb")

            # ---- PE warmup: dummy fp8 matmuls ramp the p-state while
            # the x DMAs are in flight ----
            warm_sb = sb.tile([128, 2, 2 * HW], F8, tag="warm", name="warm_sb")
            nc.gpsimd.memset(warm_sb[:], 0.0)
            # zero the energy stationary pad (read by the b=3 m-chunk)
            nc.gpsimd.memset(qk_sb[:, :, GPC * HW:], 0.0)
            wt = psum.tile([128, 2, 512], F32, tag="ps", name="warm_ps")
            for w in range(14):
                nc.tensor.matmul(
                    wt[:, w % 2, 0:2 * HW], warm_sb[:, :, 0:128],
                    warm_sb[:], start=True, stop=True, perf_mode=DR,
                )

            def tile_matmuls(pt, slot, j, gp):
                cc, jp = divmod(j, JPC)
                out = pt[:, slot, 0:2 * HW]
                ops = [(xc[cc], xc[0])]
                if (j, gp) in DIAG_TILES:
                    ops += [(xlo_sb, xc[0]), (xc[0], xlo_sb)]
                n = 2 * len(ops)
                i = 0
                for lt, rt in ops:
                    for kt in range(2):
                        nc.tensor.matmul(
                            out,
                            lt[:, 2 * kt:2 * kt + 2,
                               jp * 128:(jp + 1) * 128],
                            rt[:, 2 * kt:2 * kt + 2,
                               gp * 2 * HW:(gp + 1) * 2 * HW],
                            start=(i == 0), stop=(i == n - 1),
                            perf_mode=DR,
                        )
                        i += 1

            def j_matmuls(j):
                pt = psum.tile([128, 2, 512], F32, tag="ps", name=f"qa_{j}")
                for gp in range(2):
                    tile_matmuls(pt, gp, j, gp)
                return pt



            def seg4(t):
                return t[:, :, 0:2 * HW].rearrange("p b (s m) -> p b s m", s=2)

            def tree(srcs, out, tag):
                # srcs: list of up to 3 [128, 2, 2, 192] fp16 sbuf tiles of
                # ADJACENT j's; out: [128, 2n, 2]
                n = len(srcs)
                t1 = tpool.tile([128, 2 * n, 2, 96], F16, tag=f"t1_{n}",
                                name=f"t1_{tag}")
                for i, s in enumerate(srcs):
                    nc.vector.tensor_max(t1[:, 2 * i:2 * i + 2], s[:, :, :, 0:96],
                                         s[:, :, :, 96:192])
                t2 = tpool.tile([128, 2 * n, 2, 48], F16, tag=f"t2_{n}",
                                name=f"t2_{tag}")
                nc.vector.tensor_max(t2[:], t1[:, :, :, 0:48], t1[:, :, :, 48:96])
                t3 = tpool.tile([128, 2 * n, 2, 24], F16, tag=f"t3_{n}",
                                name=f"t3_{tag}")
                nc.vector.tensor_max(t3[:], t2[:, :, :, 0:24], t2[:, :, :, 24:48])
                nc.vector.reduce_max(out, t3[:], axis=AX_X)

            batch = []   # [(j, cst), ...] adjacent A-j's awaiting a tree

            def flush():
                if not batch:
                    return
                j0 = batch[0][0]
                n = len(batch)
                tree([c for _, c in batch],
                     res_sb[:, j0:j0 + n, :, :].rearrange(
                         "p j g s -> p (j g) s"),
                     f"j{j0}")
                batch.clear()

            def consume(j, pt):
                ap4 = seg4(pt)
                if j in _D_J:
                    nc.vector.reduce_max(res_sb[:, j, :, :], ap4, axis=AX_X)
                else:
                    if batch and batch[-1][0] != j - 1:
                        flush()
                    cst = cpool.tile([128, 2, 2, HW], F16, tag="cast",
                                     name=f"cast_{j}")
                    nc.scalar.copy(cst[:], ap4)
                    batch.append((j, cst))
                    if len(batch) == 3:
                        flush()

            def pam_proj(w_sb, qi):
                pt = psum.tile([128, 2, 512], F32, tag="ps", name=f"proj{qi}")
                for gp in range(2):
                    for kt in range(2):
                        nc.tensor.matmul(
                            pt[0:C8, gp, 0:2 * HW],
                            w_sb[:, 2 * kt:2 * kt + 2, :],
                            xc[0][:, 2 * kt:2 * kt + 2,
                                  gp * 2 * HW:(gp + 1) * 2 * HW],
                            start=(kt == 0), stop=(kt == 1), perf_mode=DR,
                        )
                return pt

            def pam_cast(pt, qi, b_sb):
                nc.scalar.activation(
                    qk_sb[:, qi, 0:2 * GROLL // 2].rearrange(
                        "p (g m) -> p g m", g=2),
                    pt[0:C8, :, 0:2 * HW], IDENT, bias=b_sb[:],
                )

            def pam_energy(half):
                et = psum.tile([128, 2, 512], F32, tag="ps", name=f"en{half}")
                for bb in range(2):
                    b = 2 * half + bb
                    for mch in range(2):
                        s = 2 * bb + mch
                        nc.tensor.matmul(
                            et[:, s // 2, (s % 2) * 256:(s % 2) * 256 + HW],
                            qk_sb[:, 0, b * HW + mch * 128:
                                  b * HW + (mch + 1) * 128],
                            qk_sb[:, 1, b * HW:(b + 1) * HW],
                            start=True, stop=True,
                        )
                return et

            def pam_finish(e0, e1):
                csts = []
                for h, et in enumerate((e0, e1)):
                    ap4 = et[:, :, :].rearrange(
                        "p b (s x) -> p b s x", s=2)[:, :, :, 0:HW]
                    cst = cpool.tile([128, 2, 2, HW], F16, tag="cast",
                                     name=f"cast_pam{h}")
                    nc.scalar.copy(cst[:], ap4)
                    csts.append(cst)
                tree(csts, pam_sb[:].rearrange("p (b s) -> p b s", b=GPC),
                     "pam")

            # ---- emission schedule ----
            # Diag j 0-5 need the (later) xlo DMA, so start with the
            # chunk-1 blocks and slot the diag blocks in once xlo lands.
            order = list(range(6, 12)) + list(range(0, 6)) + \
                list(range(12, NJ))
            proj_q = proj_k = e0 = e1 = None
            for idx, j in enumerate(order):
                pt = j_matmuls(j)
                consume(j, pt)
                if idx == 8:
                    proj_q = pam_proj(wq_sb, 0)
                    proj_k = pam_proj(wk_sb, 1)
                elif idx == 10:
                    pam_cast(proj_q, 0, bq_sb)
                    pam_cast(proj_k, 1, bk_sb)
                elif idx == 13:
                    e0 = pam_energy(0)
                elif idx == 14:
                    e1 = pam_energy(1)
                elif idx == 16:
                    pam_finish(e0, e1)
                elif idx == 20:
                    nc.sync.dma_start(pam[:], pam_sb[:])
                elif idx == 27:
                    flush()
                    # j 0..25 are consumed by now
                    nc.sync.dma_start(kres[:, 0:26, :, :],
                                      res_sb[:, 0:26, :, :])

            flush()
            nc.sync.dma_start(kres[:, 26:NJ, :, :], res_sb[:, 26:NJ, :, :])

    nc.compile()
    return nc


def kernel(x, Wq, bq, Wk, bk):
    global _COMPILED
    if _COMPILED is None:
        _COMPILED = _build()
    nc = _COMPILED

    x = np.ascontiguousarray(x, dtype=np.float32)
    xT = x.reshape(B, C, HW).transpose(1, 0, 2).reshape(C, FLAT)
    xT8 = np.ascontiguousarray(xT).astype(ml_dtypes.float8_e4m3)
    xT8f = xT8.astype(np.float32)
    wq8 = np.ascontiguousarray(np.asarray(Wq, np.float32).T).astype(
        ml_dtypes.float8_e4m3)
    wk8 = np.ascontiguousarray(np.asarray(Wk, np.float32).T).astype(
        ml_dtypes.float8_e4m3)
    bq2 = np.ascontiguousarray(np.asarray(bq, np.float32).reshape(C8, 1))
    bk2 = np.ascontiguousarray(np.asarray(bk, np.float32).reshape(C8, 1))

    in_maps = [
        {
            "xr": np.ascontiguousarray(np.roll(xT8, -i * GROLL, axis=1)),
            "xlo": np.ascontiguousarray(
                xT[:, i * GROLL:(i + 1) * GROLL]
                - xT8f[:, i * GROLL:(i + 1) * GROLL]
            ).astype(ml_dtypes.float8_e4m3),
            "wq": wq8,
            "wk": wk8,
            "bq": bq2,
            "bk": bk2,
        }
        for i in range(N_CORES)
    ]

    res = run_bass_kernel_spmd(nc, in_maps, core_ids=list(range(N_CORES)))

    kernel_max = np.empty((B, FLAT), np.float32)
    pos_max = np.empty((B, HW), np.float32)
    for i, r in enumerate(res.results):
        kr = np.asarray(r["kres"]).astype(np.float32)   # [128, NJ, 2, 2]
        arr = kr.transpose(2, 3, 1, 0).reshape(GPC, FLAT)
        for gl in range(GPC):
            kernel_max[i * GPC + gl] = np.roll(arr[gl], i * GROLL)
        pm = np.asarray(r["pam"]).astype(np.float32)    # [128, 8]
        for b in range(GPC):
            pos_max[i * GPC + b, 0:128] = pm[:, 2 * b]
            pos_max[i * GPC + b, 128:HW] = pm[0:C8, 2 * b + 1]

    return np.concatenate(
        [kernel_max.reshape(B, B, HW), pos_max[None]], axis=0
    ).astype(np.float32)
